# revision 43
# baseline (speedup 1.0000x reference)
"""Multi-head attention (B=2, S=2048, D=1024, H=16, dh=64) on 8 TRN2 NeuronCores.

Sharding: tensor-parallel over heads - 2 heads per core. Each core computes
Q/K/V projections for its 2 heads, full attention over S=2048, and a partial
output projection (its 128 rows of Wo). Host sums the 8 partial outputs + bo.

Key facts this schedule is built around (measured on this system):
  - the 8 cores share ~1.1 TB/s of HBM; every core reads the full x, so the
    input flood is the startup wall -> x is packed chunk-major (1MB pieces)
    so compute starts after the first piece, and all DMAs ride the hardware
    dynamic queues (sync/scalar). gpsimd's software-pumped DMA queue stalls
    whenever the gpsimd engine blocks, so nothing blocking goes there.
  - engines execute their queues in order, so the issue order below is a
    static pipeline schedule: exp (ACT) is the per-chunk floor (~17us), B
    feeds it densely from tt=0, C trails B by 4 t-tiles, and the previous
    chunk's normalize/output-projection plus batch-1 QKV fillers are spread
    into per-tt slots so neither ACT nor PE ever waits on the DVE chain.

Per-core dataflow:
  A) QKV:    psum[dh2=128, tok 512] = sum_k W_k[128,128].T @ x_k[128,512]
  T) V^T -> V via PE transpose (ctx matmul needs t on partitions)
  B) scoresT: psum[t=128, s 512] = K^T_h[64,128].T @ Q^T_h[64,512] (2 heads
     row-tiled, concurrent in the PE array)
  E) expT = exp(0.125 * scoresT) -> f32r (ACT, scale folded; no max-subtract)
  C) ctx aug: psum[65, 512] = sum_t [V_h|1][128,65].T @ expT[128,512]
     row 64 = softmax denominator l
  N) r = recip_approx(l) [1,1024] both heads in one DVE op; rb psum[128,512]
     via PE ones broadcast (h0 rows 0:64, h1 rows 64:128); ctxn = ctx*rb bf16
  D) out[s 128, d 512] = ctxn[:,s128][128,128].T @ Wo[128,512] -> bf16

History: baseline 424.6us -> bf16 + schedule 331 -> hardware-queue DMAs +
prepacked x 318 -> ones-columns via memset instead of the 8192-descriptor
broadcast DMA 264 -> deferred chunk tails + filler rebalance 245 -> constants
on the gpsimd queue 236 -> QKV bias-adds on ACT (DVE decongestion) + epilogue
copy split ~232-233.
"""

import numpy as np
import ml_dtypes

import concourse.bacc as bacc
import concourse.mybir as mybir
import concourse.tile as tile
from concourse.bass_utils import run_bass_kernel_spmd

F32 = mybir.dt.float32
F32R = mybir.dt.float32r
BF16 = mybir.dt.bfloat16

B, S, D, H, DH = 2, 2048, 1024, 16, 64
TOK = B * S          # 4096
DH2 = 2 * DH         # 128 (two heads per core)
NCORES = 8
SC = 512             # s-chunk
NSC = S // SC        # 4 s-chunks per batch
NT = S // 128        # 16 t-tiles per batch
NKT = D // 128       # 8 k-tiles of contraction
NCH = TOK // SC      # 8 token chunks for stage A
CLAG = 6             # C trails B/exp by this many t-tiles


def build_bass():
    nc = bacc.Bacc(None, target_bir_lowering=False)

    # x packed chunk-major on host: piece ch -> [128, kt, 512 tok] (1MB)
    xp = nc.dram_tensor("xp", [NCH, 128, NKT, SC], BF16, kind="ExternalInput")
    wqkv = nc.dram_tensor("wqkv", [128, 3, NKT, DH2], BF16,
                          kind="ExternalInput")
    bqkv = nc.dram_tensor("bqkv", [128, 3], F32, kind="ExternalInput")
    wo = nc.dram_tensor("wo", [DH2, D], BF16, kind="ExternalInput")
    onesf = nc.dram_tensor("onesf", [1, 64], BF16, kind="ExternalInput")
    iden = nc.dram_tensor("iden", [128, 128], F32, kind="ExternalInput")
    out = nc.dram_tensor("out", [TOK, D], BF16, kind="ExternalOutput")

    with tile.TileContext(nc) as tc:
        with (
            tc.tile_pool(name="persist", bufs=1) as persist,
            # one buf per x piece: a dma_start must never wait on a pool slot
            tc.tile_pool(name="xin", bufs=NCH) as xin,
            tc.tile_pool(name="exps", bufs=13) as exps,
            tc.tile_pool(name="work", bufs=2) as work,
            tc.tile_pool(name="ctxs", bufs=2) as ctxs,
            tc.tile_pool(name="ost", bufs=3) as ost,
            tc.tile_pool(name="ps_big", bufs=2, space="PSUM") as ps_big,
            tc.tile_pool(name="ps_ctx", bufs=3, space="PSUM") as ps_ctx,
            tc.tile_pool(name="ps_u", bufs=1, space="PSUM") as ps_u,
        ):
            # ---- constants / persistent tiles ----
            # k-weights first (first A work), q next; bulky wo/ident later
            w_sb = persist.tile([128, 3, NKT, DH2], BF16, tag="w")
            b_sb = persist.tile([128, 3], F32, tag="b")
            nc.gpsimd.dma_start(out=b_sb[:], in_=bqkv[:, :])
            wo_sb = persist.tile([128, D], BF16, tag="wo")
            ident = persist.tile([128, 128], F32R, tag="id")
            ones1 = persist.tile([1, 64], BF16, tag="o1")
            nc.gpsimd.dma_start(out=ones1[:], in_=onesf[:, :])

            qT = persist.tile([128, TOK], BF16, tag="qT")
            kT = persist.tile([128, TOK], BF16, tag="kT")
            vT = persist.tile([128, TOK], F32R, tag="vT")
            # V in [t, e] layout, 130 = [V_h0(64) | 1 | V_h1(64) | 1]
            v_sb = persist.tile([128, TOK // 128, 130], F32R, tag="v")
            # ones columns via DVE memset: a stride-0 broadcast DMA would
            # shatter into 8192 4-byte software-queue descriptors (~92us!)
            nc.vector.memset(v_sb[:, :, 64].bitcast(F32), 1.0)
            nc.vector.memset(v_sb[:, :, 129].bitcast(F32), 1.0)

            # ---------------- stage helpers ----------------
            xtiles = {}

            def issue_x_dmas(chs, engs):
                for i, ch in enumerate(chs):
                    x_t = xin.tile([128, NKT, SC], BF16, tag="x",
                                   name=f"x{ch}")
                    engs[i % len(engs)].dma_start(out=x_t[:], in_=xp[ch])
                    xtiles[ch] = x_t

            def stage_a_proj(ch, p):
                """One projection (0=q,1=k,2=v) for token chunk ch."""
                c0 = ch * SC
                dests = (qT, kT, vT)
                ps_p = ps_u.tile([128, SC], F32, tag="u")
                for kt in range(NKT):
                    nc.tensor.matmul(
                        ps_p[:],
                        w_sb[:, p, kt, :],
                        xtiles[ch][:, kt, :],
                        start=(kt == 0), stop=(kt == NKT - 1),
                    )
                # bias-adds ride ACT (Identity shares the exp table set):
                # keeps the DVE queue clear, whose backlog gates each next
                # chunk's first B via the q bias-add
                nc.scalar.activation(
                    dests[p][:, c0:c0 + SC], ps_p[:],
                    mybir.ActivationFunctionType.Identity,
                    bias=b_sb[:, p:p + 1],
                )

            def stage_t_blk2(blk):
                """Transpose two 128-col blocks of V^T into v_sb with one
                ps_u cycle."""
                ps_t = ps_u.tile([128, 256], F32R, tag="u", name="ps_t")
                for j in range(2):
                    b0 = blk + j
                    nc.tensor.transpose(
                        ps_t[:, j * 128:(j + 1) * 128],
                        vT[:, b0 * 128:(b0 + 1) * 128], ident[:]
                    )
                for j in range(2):
                    b0 = blk + j
                    nc.vector.tensor_copy(
                        v_sb[:, b0, 0:64], ps_t[:, j * 128:j * 128 + 64])
                    nc.vector.tensor_copy(
                        v_sb[:, b0, 65:129],
                        ps_t[:, j * 128 + 64:(j + 1) * 128])

            # deferred tail state from the previous chunk
            pend = {}

            def issue_rb(st):
                """PE broadcast of r rows into ps_rb, then ctxn muls (DVE)."""
                ps_rb = ps_u.tile([128, SC], F32, tag="u")
                for h in range(2):
                    nc.tensor.matmul(
                        ps_rb[h * 64:(h + 1) * 64, :],
                        ones1[:],
                        st["r2"][h],
                        start=True, stop=True,
                    )
                rb_sb = work.tile([128, SC], F32, tag="rb")
                nc.vector.tensor_copy(rb_sb[:], ps_rb[:])
                ctxn = ctxs.tile([128, SC], BF16, tag="ctxn")
                for h in range(2):
                    nc.vector.tensor_mul(
                        ctxn[h * 64:(h + 1) * 64, :],
                        st["ps_c"][h][0:64, :],
                        rb_sb[h * 64:(h + 1) * 64, :],
                    )
                st["ctxn"] = ctxn

            def issue_d_half(st, ss, dc, pool=None):
                """Half an s-subtile (one 512-wide dc) of the output
                projection of a pending chunk. Mid-loop: copies on DVE, DMA
                fires on dc=1. Epilogue (pool set): copies alternate between
                DVE and the now-idle ACT, and each half DMAs immediately."""
                q0 = st["q0"]
                ctxn = st["ctxn"]
                if dc == 0:
                    st["o_sb"] = ost.tile([128, 1024], BF16, tag="o",
                                          name="o_sb")
                o_sb = st["o_sb"]
                ps_o = (ps_u if pool is None else pool).tile(
                    [128, SC], F32, tag="u" if pool is None else "ctx",
                    name="ps_o")
                nc.tensor.matmul(
                    ps_o[:],
                    ctxn[:, ss * 128:(ss + 1) * 128],
                    wo_sb[:, dc * SC:(dc + 1) * SC],
                    start=True, stop=True,
                )
                dst = o_sb[:, dc * SC:(dc + 1) * SC]
                if pool is None:
                    nc.vector.tensor_copy(dst, ps_o[:])
                    if dc == 1:
                        nc.sync.dma_start(
                            out=out[q0 + ss * 128:q0 + (ss + 1) * 128, :],
                            in_=o_sb[:]
                        )
                else:
                    if (ss + dc) % 2 == 0:
                        nc.vector.tensor_copy(dst, ps_o[:])
                    else:
                        nc.scalar.copy(dst, ps_o[:])
                    nc.sync.dma_start(
                        out=out[q0 + ss * 128:q0 + (ss + 1) * 128,
                                dc * SC:(dc + 1) * SC],
                        in_=dst,
                    )

            def issue_d(st, ss, pool=None):
                for dc in range(2):
                    issue_d_half(st, ss, dc, pool=pool)

            def issue_cdrain(st, lags):
                """Deferred tail C-accumulations of the previous chunk."""
                for lag in lags:
                    for h in range(2):
                        nc.tensor.matmul(
                            st["ps_c"][h][:],
                            v_sb[:, st["bt"] + lag, h * 65:h * 65 + 65],
                            st["etiles"][lag - 12][:, h * SC:(h + 1) * SC],
                            start=False, stop=(lag == NT - 1),
                        )

            def issue_lrecip(st):
                """Denominator rows -> 1/l (DVE), bf16-cast for the rb MM."""
                l2 = work.tile([1, 2 * SC], F32, tag="l2", name="l2")
                for h in range(2):
                    nc.vector.tensor_copy(l2[0:1, h * SC:(h + 1) * SC],
                                          st["ps_c"][h][64:65, :])
                r2h = work.tile([1, 2 * SC], F32, tag="r2", name="r2")
                nc.vector.reciprocal_approx_fast(r2h[:], l2[:])
                r2r = work.tile([1, 2 * SC], BF16, tag="r2r", name="r2r")
                nc.vector.tensor_copy(r2r[:], r2h[:])
                st["r2"] = [r2r[0:1, 0:SC], r2r[0:1, SC:2 * SC]]

            LAST_EXTRA = {13: (10,), 14: (11, 12), 15: (13, 14)}

            def issue_b(b, sc, tt):
                """One scores pair + exp for t-tile tt of chunk (b, sc)."""
                q0 = b * S + sc * SC
                t0 = b * S + tt * 128
                ps_s = ps_big.tile([128, 1024], F32, tag="big", name="ps_s")
                nc.tensor.matmul(
                    ps_s[:, 0:SC],
                    kT[0:64, t0:t0 + 128],
                    qT[0:64, q0:q0 + SC],
                    start=True, stop=True,
                )
                nc.tensor.matmul(
                    ps_s[:, SC:2 * SC],
                    kT[64:128, t0:t0 + 128],
                    qT[64:128, q0:q0 + SC],
                    start=True, stop=True,
                )
                e_t = exps.tile([128, 1024], F32R, tag="e", name="e_t")
                nc.scalar.activation(
                    e_t[:], ps_s[:],
                    mybir.ActivationFunctionType.Exp, scale=0.125,
                )
                return e_t

            def chunk_body(b, sc, fillers, last=False, nxt=None):
                """Every chunk's first two B/exp pairs are prefetched by the
                previous chunk (`nxt` chaining) so exp never starves at a
                boundary; the previous chunk's tail (C lags 12-15, l/recip,
                rb/ctxn, D) executes in early tt slots and odd lags. C trails
                B/exp by CLAG=6."""
                q0 = b * S + sc * SC
                st_prev = pend.pop("st", None)
                etiles = pend.pop("hand", [])
                ps_c = None
                for tt in range(len(etiles), NT):
                    etiles.append(issue_b(b, sc, tt))
                    if st_prev is not None:
                        if tt == 2:
                            issue_cdrain(st_prev, (12, 13, 14, 15))
                        elif tt == 3:
                            issue_lrecip(st_prev)
                        elif tt == 4:
                            issue_rb(st_prev)
                    if tt == CLAG - 1:
                        ps_c = [
                            ps_ctx.tile([65, SC], F32, tag="ctx",
                                        name=f"psc{h}")
                            for h in range(2)
                        ]
                    def issue_c(lag):
                        for h in range(2):
                            nc.tensor.matmul(
                                ps_c[h][:],
                                v_sb[:, b * NT + lag, h * 65:h * 65 + 65],
                                etiles[lag][:, h * SC:(h + 1) * SC],
                                start=(lag == 0), stop=(lag == NT - 1),
                            )
                    if tt >= CLAG:
                        issue_c(tt - CLAG)
                        lag = tt - CLAG
                        if st_prev is not None and 1 <= lag <= 8:
                            issue_d_half(st_prev, (lag - 1) // 2,
                                         (lag - 1) % 2)
                    if last:
                        for lag in LAST_EXTRA.get(tt, ()):
                            issue_c(lag)
                    for f in fillers.get(tt, ()):
                        f()
                if not last:
                    if nxt is not None:
                        pend["hand"] = [issue_b(nxt[0], nxt[1], 0),
                                        issue_b(nxt[0], nxt[1], 1)]
                    for lag in range(NT - CLAG, NT - 4):
                        issue_c(lag)
                    pend["st"] = {"q0": q0, "ps_c": ps_c, "bt": b * NT,
                                  "etiles": etiles[12:16]}
                else:
                    issue_c(NT - 1)
                    st = {"q0": q0, "ps_c": ps_c, "bt": b * NT}
                    issue_lrecip(st)
                    issue_rb(st)
                    for ss in range(4):
                        issue_d(st, ss, pool=ps_ctx)

            # ---------------- issue order ----------------
            # A-projection and V-transpose work is threaded into per-tt
            # filler slots so it pipelines with B/exp/C and the x flood.
            def A(ch, p):
                return lambda: stage_a_proj(ch, p)

            def T(blk):
                return lambda: stage_t_blk2(blk)

            # piece 0 split across both queues; k-weights parallel it
            x0 = xin.tile([128, NKT, SC], BF16, tag="x", name="x0")
            nc.sync.dma_start(out=x0[:, 0:4, :], in_=xp[0][:, 0:4, :])
            nc.scalar.dma_start(out=x0[:, 4:8, :], in_=xp[0][:, 4:8, :])
            xtiles[0] = x0
            nc.sync.dma_start(out=w_sb[:, 1], in_=wqkv[:, 1])
            nc.scalar.dma_start(out=w_sb[:, 0], in_=wqkv[:, 0])
            # w_v/wo/ident ride the (otherwise idle, never-blocked) gpsimd
            # software queue so the x pieces own the hardware queues
            nc.gpsimd.dma_start(out=w_sb[:, 2], in_=wqkv[:, 2])
            nc.gpsimd.dma_start(out=wo_sb[:], in_=wo[:, :])
            nc.gpsimd.dma_start(out=ident[:], in_=iden[:, :].bitcast(F32R))
            issue_x_dmas(range(1, NCH), [nc.sync, nc.scalar])
            stage_a_proj(0, 1)  # k(ch0)
            stage_a_proj(0, 0)  # q(ch0)

            chunk_body(0, 0, {
                1: [A(0, 2), A(1, 1)], 2: [T(0)], 3: [T(2)],
                4: [A(1, 2), A(2, 1)], 5: [T(4)], 6: [T(6)],
                7: [A(2, 2), A(3, 1)], 8: [T(8)], 9: [T(10)],
                10: [A(3, 2), A(1, 0)], 11: [T(12)], 12: [T(14)],
            }, nxt=(0, 1))
            chunk_body(0, 1, {4: [A(4, 1)], 5: [A(4, 2)], 7: [A(2, 0)]},
                       nxt=(0, 2))
            chunk_body(0, 2, {4: [A(5, 1)], 5: [A(5, 2)], 7: [A(3, 0)],
                              8: [T(16)], 9: [T(18)]}, nxt=(0, 3))
            chunk_body(0, 3, {4: [A(6, 1)], 5: [A(6, 2)], 7: [A(4, 0)],
                              8: [T(20)], 9: [T(22)]}, nxt=(1, 0))
            chunk_body(1, 0, {4: [A(7, 1)], 5: [A(7, 2)], 6: [A(5, 0)],
                              7: [T(24)], 8: [T(26)], 9: [T(28)],
                              10: [T(30)]}, nxt=(1, 1))
            chunk_body(1, 1, {4: [A(6, 0)]}, nxt=(1, 2))
            chunk_body(1, 2, {4: [A(7, 0)]}, nxt=(1, 3))
            chunk_body(1, 3, {}, last=True)
    nc.finalize()
    return nc


_NC_CACHE = None


def make_in_maps(x, Wq, Wk, Wv, bq, bk, bv, Wo, bo=None):
    bf = ml_dtypes.bfloat16
    xT = x.reshape(TOK, D).T.astype(bf)  # [D, TOK]
    # piece ch -> [128 p, kt, 512 tok]: xT[kt*128+p, ch*512+c]
    xp = np.ascontiguousarray(
        xT.reshape(NKT, 128, NCH, SC).transpose(2, 1, 0, 3))

    def pack_w(w2):  # [D, DH2] -> [128, NKT, DH2]
        return w2.reshape(NKT, 128, DH2).transpose(1, 0, 2)

    in_maps = []
    for c in range(NCORES):
        h0 = 2 * c
        wq2 = np.concatenate([Wq[h0], Wq[h0 + 1]], axis=1)
        wk2 = np.concatenate([Wk[h0], Wk[h0 + 1]], axis=1)
        wv2 = np.concatenate([Wv[h0], Wv[h0 + 1]], axis=1)
        wqkv = np.ascontiguousarray(
            np.stack([pack_w(wq2), pack_w(wk2), pack_w(wv2)], axis=1)
        ).astype(bf)
        bqkv = np.ascontiguousarray(np.stack([
            bq[h0:h0 + 2].reshape(DH2),
            bk[h0:h0 + 2].reshape(DH2),
            bv[h0:h0 + 2].reshape(DH2),
        ], axis=1)).astype(np.float32)
        in_maps.append({
            "xp": xp,
            "wqkv": wqkv,
            "bqkv": bqkv,
            "wo": np.ascontiguousarray(Wo[c * DH2:(c + 1) * DH2]).astype(bf),
            "onesf": np.ones((1, 64), dtype=bf),
            "iden": np.eye(128, dtype=np.float32),
        })
    return in_maps


def kernel(x, Wq, Wk, Wv, bq, bk, bv, Wo, bo):
    global _NC_CACHE
    if _NC_CACHE is None:
        _NC_CACHE = build_bass()
    nc = _NC_CACHE

    in_maps = make_in_maps(x, Wq, Wk, Wv, bq, bk, bv, Wo)
    res = run_bass_kernel_spmd(nc, in_maps, list(range(NCORES)))
    acc = np.zeros((TOK, D), dtype=np.float64)
    for c in range(NCORES):
        acc += res.results[c]["out"].astype(np.float64)
    acc += bo
    return acc.astype(np.float32).reshape(B, S, D)


# revision 45
# speedup vs baseline: 1.0128x; 1.0128x over previous
"""Multi-head attention (B=2, S=2048, D=1024, H=16, dh=64) on 8 TRN2 NeuronCores.

Sharding: tensor-parallel over heads - 2 heads per core. Each core computes
Q/K/V projections for its 2 heads, full attention over S=2048, and a partial
output projection (its 128 rows of Wo). Host sums the 8 partial outputs + bo.

Key facts this schedule is built around (measured on this system):
  - the 8 cores share ~1.1 TB/s of HBM; every core reads the full x, so the
    input flood is the startup wall -> x is packed chunk-major (1MB pieces)
    so compute starts after the first piece, and all DMAs ride the hardware
    dynamic queues (sync/scalar). gpsimd's software-pumped DMA queue stalls
    whenever the gpsimd engine blocks, so nothing blocking goes there.
  - engines execute their queues in order, so the issue order below is a
    static pipeline schedule: exp (ACT) is the per-chunk floor (~17us), B
    feeds it densely from tt=0, C trails B by 4 t-tiles, and the previous
    chunk's normalize/output-projection plus batch-1 QKV fillers are spread
    into per-tt slots so neither ACT nor PE ever waits on the DVE chain.

Per-core dataflow:
  A) QKV:    psum[dh2=128, tok 512] = sum_k W_k[128,128].T @ x_k[128,512]
  T) V^T -> V via PE transpose (ctx matmul needs t on partitions)
  B) scoresT: psum[t=128, s 512] = K^T_h[64,128].T @ Q^T_h[64,512] (2 heads
     row-tiled, concurrent in the PE array)
  E) expT = exp(0.125 * scoresT) -> f32r (ACT, scale folded; no max-subtract)
  C) ctx aug: psum[65, 512] = sum_t [V_h|1][128,65].T @ expT[128,512]
     row 64 = softmax denominator l
  N) r = recip_approx(l) [1,1024] both heads in one DVE op; rb psum[128,512]
     via PE ones broadcast (h0 rows 0:64, h1 rows 64:128); ctxn = ctx*rb bf16
  D) out[s 128, d 512] = ctxn[:,s128][128,128].T @ Wo[128,512] -> bf16

History: baseline 424.6us -> bf16 + schedule 331 -> hardware-queue DMAs +
prepacked x 318 -> ones-columns via memset instead of the 8192-descriptor
broadcast DMA 264 -> deferred chunk tails + filler rebalance 245 -> constants
on the gpsimd software queue 236 -> QKV bias-adds on ACT via Identity
(decongests the DVE queue that gates each next chunk's first B) ~232-235.
"""

import numpy as np
import ml_dtypes

import concourse.bacc as bacc
import concourse.mybir as mybir
import concourse.tile as tile
from concourse.bass_utils import run_bass_kernel_spmd

F32 = mybir.dt.float32
F32R = mybir.dt.float32r
BF16 = mybir.dt.bfloat16

B, S, D, H, DH = 2, 2048, 1024, 16, 64
TOK = B * S          # 4096
DH2 = 2 * DH         # 128 (two heads per core)
NCORES = 8
SC = 512             # s-chunk
NSC = S // SC        # 4 s-chunks per batch
NT = S // 128        # 16 t-tiles per batch
NKT = D // 128       # 8 k-tiles of contraction
NCH = TOK // SC      # 8 token chunks for stage A
CLAG = 6             # C trails B/exp by this many t-tiles


def build_bass():
    nc = bacc.Bacc(None, target_bir_lowering=False)

    # x packed chunk-major on host: piece ch -> [128, kt, 512 tok] (1MB)
    xp = nc.dram_tensor("xp", [NCH, 128, NKT, SC], BF16, kind="ExternalInput")
    wqkv = nc.dram_tensor("wqkv", [128, 3, NKT, DH2], BF16,
                          kind="ExternalInput")
    bqkv = nc.dram_tensor("bqkv", [128, 3], F32, kind="ExternalInput")
    wo = nc.dram_tensor("wo", [DH2, D], BF16, kind="ExternalInput")
    onesf = nc.dram_tensor("onesf", [1, 64], BF16, kind="ExternalInput")
    iden = nc.dram_tensor("iden", [128, 128], F32, kind="ExternalInput")
    out = nc.dram_tensor("out", [TOK, D], BF16, kind="ExternalOutput")

    with tile.TileContext(nc) as tc:
        with (
            tc.tile_pool(name="persist", bufs=1) as persist,
            # one buf per x piece: a dma_start must never wait on a pool slot
            tc.tile_pool(name="xin", bufs=NCH) as xin,
            tc.tile_pool(name="exps", bufs=13) as exps,
            tc.tile_pool(name="work", bufs=2) as work,
            tc.tile_pool(name="ctxs", bufs=2) as ctxs,
            tc.tile_pool(name="ost", bufs=3) as ost,
            tc.tile_pool(name="ps_big", bufs=2, space="PSUM") as ps_big,
            tc.tile_pool(name="ps_ctx", bufs=3, space="PSUM") as ps_ctx,
            tc.tile_pool(name="ps_u", bufs=1, space="PSUM") as ps_u,
        ):
            # ---- constants / persistent tiles ----
            # k-weights first (first A work), q next; bulky wo/ident later
            w_sb = persist.tile([128, 3, NKT, DH2], BF16, tag="w")
            b_sb = persist.tile([128, 3], F32, tag="b")
            nc.gpsimd.dma_start(out=b_sb[:], in_=bqkv[:, :])
            wo_sb = persist.tile([128, D], BF16, tag="wo")
            ident = persist.tile([128, 128], F32R, tag="id")
            ones1 = persist.tile([1, 64], BF16, tag="o1")
            nc.gpsimd.dma_start(out=ones1[:], in_=onesf[:, :])

            qT = persist.tile([128, TOK], BF16, tag="qT")
            kT = persist.tile([128, TOK], BF16, tag="kT")
            vT = persist.tile([128, TOK], F32R, tag="vT")
            # V in [t, e] layout, 130 = [V_h0(64) | 1 | V_h1(64) | 1]
            v_sb = persist.tile([128, TOK // 128, 130], F32R, tag="v")
            # ones columns via DVE memset: a stride-0 broadcast DMA would
            # shatter into 8192 4-byte software-queue descriptors (~92us!)
            nc.vector.memset(v_sb[:, :, 64].bitcast(F32), 1.0)
            nc.vector.memset(v_sb[:, :, 129].bitcast(F32), 1.0)

            # ---------------- stage helpers ----------------
            xtiles = {}

            def issue_x_dmas(chs, engs):
                for i, ch in enumerate(chs):
                    x_t = xin.tile([128, NKT, SC], BF16, tag="x",
                                   name=f"x{ch}")
                    engs[i % len(engs)].dma_start(out=x_t[:], in_=xp[ch])
                    xtiles[ch] = x_t

            def stage_a_proj(ch, p):
                """One projection (0=q,1=k,2=v) for token chunk ch."""
                c0 = ch * SC
                dests = (qT, kT, vT)
                ps_p = ps_u.tile([128, SC], F32, tag="u")
                for kt in range(NKT):
                    nc.tensor.matmul(
                        ps_p[:],
                        w_sb[:, p, kt, :],
                        xtiles[ch][:, kt, :],
                        start=(kt == 0), stop=(kt == NKT - 1),
                    )
                # bias-adds ride ACT (Identity shares the exp table set):
                # keeps the DVE queue clear, whose backlog gates each next
                # chunk's first B via the q bias-add
                nc.scalar.activation(
                    dests[p][:, c0:c0 + SC], ps_p[:],
                    mybir.ActivationFunctionType.Identity,
                    bias=b_sb[:, p:p + 1],
                )

            def stage_t_blk2(blk):
                """Transpose two 128-col blocks of V^T into v_sb with one
                ps_u cycle."""
                ps_t = ps_u.tile([128, 256], F32R, tag="u", name="ps_t")
                for j in range(2):
                    b0 = blk + j
                    nc.tensor.transpose(
                        ps_t[:, j * 128:(j + 1) * 128],
                        vT[:, b0 * 128:(b0 + 1) * 128], ident[:]
                    )
                for j in range(2):
                    b0 = blk + j
                    nc.vector.tensor_copy(
                        v_sb[:, b0, 0:64], ps_t[:, j * 128:j * 128 + 64])
                    nc.vector.tensor_copy(
                        v_sb[:, b0, 65:129],
                        ps_t[:, j * 128 + 64:(j + 1) * 128])

            # deferred tail state from the previous chunk
            pend = {}

            def issue_rb(st):
                """PE broadcast of r rows into ps_rb, then ctxn muls (DVE)."""
                ps_rb = ps_u.tile([128, SC], F32, tag="u")
                for h in range(2):
                    nc.tensor.matmul(
                        ps_rb[h * 64:(h + 1) * 64, :],
                        ones1[:],
                        st["r2"][h],
                        start=True, stop=True,
                    )
                rb_sb = work.tile([128, SC], F32, tag="rb")
                nc.vector.tensor_copy(rb_sb[:], ps_rb[:])
                ctxn = ctxs.tile([128, SC], BF16, tag="ctxn")
                for h in range(2):
                    nc.vector.tensor_mul(
                        ctxn[h * 64:(h + 1) * 64, :],
                        st["ps_c"][h][0:64, :],
                        rb_sb[h * 64:(h + 1) * 64, :],
                    )
                st["ctxn"] = ctxn

            def issue_d_half(st, ss, dc, pool=None):
                """Half an s-subtile (one 512-wide dc) of the output
                projection of a pending chunk; the out DMA fires on dc=1."""
                q0 = st["q0"]
                ctxn = st["ctxn"]
                if dc == 0:
                    st["o_sb"] = ost.tile([128, 1024], BF16, tag="o",
                                          name="o_sb")
                o_sb = st["o_sb"]
                ps_o = (ps_u if pool is None else pool).tile(
                    [128, SC], F32, tag="u" if pool is None else "ctx",
                    name="ps_o")
                nc.tensor.matmul(
                    ps_o[:],
                    ctxn[:, ss * 128:(ss + 1) * 128],
                    wo_sb[:, dc * SC:(dc + 1) * SC],
                    start=True, stop=True,
                )
                nc.vector.tensor_copy(o_sb[:, dc * SC:(dc + 1) * SC], ps_o[:])
                if dc == 1:
                    nc.sync.dma_start(
                        out=out[q0 + ss * 128:q0 + (ss + 1) * 128, :],
                        in_=o_sb[:]
                    )

            def issue_d(st, ss, pool=None):
                for dc in range(2):
                    issue_d_half(st, ss, dc, pool=pool)

            def issue_cdrain(st, lags):
                """Deferred tail C-accumulations of the previous chunk."""
                for lag in lags:
                    for h in range(2):
                        nc.tensor.matmul(
                            st["ps_c"][h][:],
                            v_sb[:, st["bt"] + lag, h * 65:h * 65 + 65],
                            st["etiles"][lag - 12][:, h * SC:(h + 1) * SC],
                            start=False, stop=(lag == NT - 1),
                        )

            def issue_lrecip(st):
                """Denominator rows -> 1/l (DVE), bf16-cast for the rb MM."""
                l2 = work.tile([1, 2 * SC], F32, tag="l2", name="l2")
                for h in range(2):
                    nc.vector.tensor_copy(l2[0:1, h * SC:(h + 1) * SC],
                                          st["ps_c"][h][64:65, :])
                r2h = work.tile([1, 2 * SC], F32, tag="r2", name="r2")
                nc.vector.reciprocal_approx_fast(r2h[:], l2[:])
                r2r = work.tile([1, 2 * SC], BF16, tag="r2r", name="r2r")
                nc.vector.tensor_copy(r2r[:], r2h[:])
                st["r2"] = [r2r[0:1, 0:SC], r2r[0:1, SC:2 * SC]]

            LAST_EXTRA = {13: (10,), 14: (11, 12), 15: (13, 14)}

            def issue_b(b, sc, tt):
                """One scores pair + exp for t-tile tt of chunk (b, sc)."""
                q0 = b * S + sc * SC
                t0 = b * S + tt * 128
                ps_s = ps_big.tile([128, 1024], F32, tag="big", name="ps_s")
                nc.tensor.matmul(
                    ps_s[:, 0:SC],
                    kT[0:64, t0:t0 + 128],
                    qT[0:64, q0:q0 + SC],
                    start=True, stop=True,
                )
                nc.tensor.matmul(
                    ps_s[:, SC:2 * SC],
                    kT[64:128, t0:t0 + 128],
                    qT[64:128, q0:q0 + SC],
                    start=True, stop=True,
                )
                e_t = exps.tile([128, 1024], F32R, tag="e", name="e_t")
                nc.scalar.activation(
                    e_t[:], ps_s[:],
                    mybir.ActivationFunctionType.Exp, scale=0.125,
                )
                return e_t

            def chunk_body(b, sc, fillers, last=False, nxt=None):
                """Every chunk's first two B/exp pairs are prefetched by the
                previous chunk (`nxt` chaining) so exp never starves at a
                boundary; the previous chunk's tail (C lags 12-15, l/recip,
                rb/ctxn, D) executes in early tt slots and odd lags. C trails
                B/exp by CLAG=6."""
                q0 = b * S + sc * SC
                st_prev = pend.pop("st", None)
                etiles = pend.pop("hand", [])
                ps_c = None
                for tt in range(len(etiles), NT):
                    etiles.append(issue_b(b, sc, tt))
                    if st_prev is not None:
                        if tt == 2:
                            issue_cdrain(st_prev, (12, 13, 14, 15))
                        elif tt == 3:
                            issue_lrecip(st_prev)
                        elif tt == 4:
                            issue_rb(st_prev)
                    if tt == CLAG - 1:
                        ps_c = [
                            ps_ctx.tile([65, SC], F32, tag="ctx",
                                        name=f"psc{h}")
                            for h in range(2)
                        ]
                    def issue_c(lag):
                        for h in range(2):
                            nc.tensor.matmul(
                                ps_c[h][:],
                                v_sb[:, b * NT + lag, h * 65:h * 65 + 65],
                                etiles[lag][:, h * SC:(h + 1) * SC],
                                start=(lag == 0), stop=(lag == NT - 1),
                            )
                    if tt >= CLAG:
                        issue_c(tt - CLAG)
                        lag = tt - CLAG
                        if st_prev is not None and 1 <= lag <= 8:
                            issue_d_half(st_prev, (lag - 1) // 2,
                                         (lag - 1) % 2)
                    if last:
                        for lag in LAST_EXTRA.get(tt, ()):
                            issue_c(lag)
                    for f in fillers.get(tt, ()):
                        f()
                if not last:
                    if nxt is not None:
                        pend["hand"] = [issue_b(nxt[0], nxt[1], 0),
                                        issue_b(nxt[0], nxt[1], 1)]
                    for lag in range(NT - CLAG, NT - 4):
                        issue_c(lag)
                    pend["st"] = {"q0": q0, "ps_c": ps_c, "bt": b * NT,
                                  "etiles": etiles[12:16]}
                else:
                    issue_c(NT - 1)
                    st = {"q0": q0, "ps_c": ps_c, "bt": b * NT}
                    issue_lrecip(st)

                    def keep_warm(name):
                        # dummy transpose: PE stays busy through the DVE
                        # normalize chain so HAM never re-throttles the clock
                        wt = ps_ctx.tile([128, 128], F32R, tag="ctx",
                                         name=name)
                        nc.tensor.transpose(wt[:], vT[:, 0:128], ident[:])
                    keep_warm("warm1")
                    issue_rb(st)
                    keep_warm("warm2")
                    for ss in range(4):
                        issue_d(st, ss, pool=ps_ctx)

            # ---------------- issue order ----------------
            # A-projection and V-transpose work is threaded into per-tt
            # filler slots so it pipelines with B/exp/C and the x flood.
            def A(ch, p):
                return lambda: stage_a_proj(ch, p)

            def T(blk):
                return lambda: stage_t_blk2(blk)

            # piece 0 split across both queues; k-weights parallel it
            x0 = xin.tile([128, NKT, SC], BF16, tag="x", name="x0")
            nc.sync.dma_start(out=x0[:, 0:4, :], in_=xp[0][:, 0:4, :])
            nc.scalar.dma_start(out=x0[:, 4:8, :], in_=xp[0][:, 4:8, :])
            xtiles[0] = x0
            nc.sync.dma_start(out=w_sb[:, 1], in_=wqkv[:, 1])
            nc.scalar.dma_start(out=w_sb[:, 0], in_=wqkv[:, 0])
            # w_v/wo/ident ride the (otherwise idle, never-blocked) gpsimd
            # software queue so the x pieces own the hardware queues
            nc.gpsimd.dma_start(out=w_sb[:, 2], in_=wqkv[:, 2])
            nc.gpsimd.dma_start(out=wo_sb[:], in_=wo[:, :])
            nc.gpsimd.dma_start(out=ident[:], in_=iden[:, :].bitcast(F32R))
            issue_x_dmas(range(1, NCH), [nc.sync, nc.scalar])
            stage_a_proj(0, 1)  # k(ch0)
            stage_a_proj(0, 0)  # q(ch0)

            chunk_body(0, 0, {
                1: [A(0, 2), A(1, 1)], 2: [T(0)], 3: [T(2)],
                4: [A(1, 2), A(2, 1)], 5: [T(4)], 6: [T(6)],
                7: [A(2, 2), A(3, 1)], 8: [T(8)], 9: [T(10)],
                10: [A(3, 2), A(1, 0)], 11: [T(12)], 12: [T(14)],
            }, nxt=(0, 1))
            chunk_body(0, 1, {4: [A(4, 1)], 5: [A(4, 2)], 7: [A(2, 0)]},
                       nxt=(0, 2))
            chunk_body(0, 2, {4: [A(5, 1)], 5: [A(5, 2)], 7: [A(3, 0)],
                              8: [T(16)], 9: [T(18)]}, nxt=(0, 3))
            chunk_body(0, 3, {4: [A(6, 1)], 5: [A(6, 2)], 7: [A(4, 0)],
                              8: [T(20)], 9: [T(22)]}, nxt=(1, 0))
            chunk_body(1, 0, {4: [A(7, 1)], 5: [A(7, 2)], 6: [A(5, 0)],
                              7: [T(24)], 8: [T(26)], 9: [T(28)],
                              10: [T(30)]}, nxt=(1, 1))
            chunk_body(1, 1, {4: [A(6, 0)]}, nxt=(1, 2))
            chunk_body(1, 2, {4: [A(7, 0)]}, nxt=(1, 3))
            chunk_body(1, 3, {}, last=True)
    nc.finalize()
    return nc


_NC_CACHE = None


def make_in_maps(x, Wq, Wk, Wv, bq, bk, bv, Wo, bo=None):
    bf = ml_dtypes.bfloat16
    xT = x.reshape(TOK, D).T.astype(bf)  # [D, TOK]
    # piece ch -> [128 p, kt, 512 tok]: xT[kt*128+p, ch*512+c]
    xp = np.ascontiguousarray(
        xT.reshape(NKT, 128, NCH, SC).transpose(2, 1, 0, 3))

    def pack_w(w2):  # [D, DH2] -> [128, NKT, DH2]
        return w2.reshape(NKT, 128, DH2).transpose(1, 0, 2)

    in_maps = []
    for c in range(NCORES):
        h0 = 2 * c
        wq2 = np.concatenate([Wq[h0], Wq[h0 + 1]], axis=1)
        wk2 = np.concatenate([Wk[h0], Wk[h0 + 1]], axis=1)
        wv2 = np.concatenate([Wv[h0], Wv[h0 + 1]], axis=1)
        wqkv = np.ascontiguousarray(
            np.stack([pack_w(wq2), pack_w(wk2), pack_w(wv2)], axis=1)
        ).astype(bf)
        bqkv = np.ascontiguousarray(np.stack([
            bq[h0:h0 + 2].reshape(DH2),
            bk[h0:h0 + 2].reshape(DH2),
            bv[h0:h0 + 2].reshape(DH2),
        ], axis=1)).astype(np.float32)
        in_maps.append({
            "xp": xp,
            "wqkv": wqkv,
            "bqkv": bqkv,
            "wo": np.ascontiguousarray(Wo[c * DH2:(c + 1) * DH2]).astype(bf),
            "onesf": np.ones((1, 64), dtype=bf),
            "iden": np.eye(128, dtype=np.float32),
        })
    return in_maps


def kernel(x, Wq, Wk, Wv, bq, bk, bv, Wo, bo):
    global _NC_CACHE
    if _NC_CACHE is None:
        _NC_CACHE = build_bass()
    nc = _NC_CACHE

    in_maps = make_in_maps(x, Wq, Wk, Wv, bq, bk, bv, Wo)
    res = run_bass_kernel_spmd(nc, in_maps, list(range(NCORES)))
    acc = np.zeros((TOK, D), dtype=np.float64)
    for c in range(NCORES):
        acc += res.results[c]["out"].astype(np.float64)
    acc += bo
    return acc.astype(np.float32).reshape(B, S, D)
